# revision 60
# baseline (speedup 1.0000x reference)
"""Attention-pooling kernel (AttLayer) for Trainium2, 8 NeuronCores.

Math (per batch b):
    uit  = tanh(x @ W + b)          # [T, A]
    e    = exp(uit @ u)             # [T]
    out  = (sum_t e[t] * x[t,:]) / (sum_t e[t] + EPS)   # [D]

Per-core structure (pure data parallel over batch, BL=8 batches/core),
processing halves of T (TH=2048) so every engine streams concurrently:

    PE  : ps_uitT[100, 1024] = W^T @ x-half   (both 1024-quarters of the
          half packed on the partition axis: q0 -> rows 0-49, q1 -> 50-99;
          matmuls grouped per stationary: LDW w0 x4MM, LDW w1 x4MM)
    ACT : uitT = tanh(ps_uitT + bias2) as ONE [100, 1024] instr
    PE  : ps_logit[128, 1024] = urep^T @ uitT[rows]  per quarter
          (128 identical rows = partition-broadcast of the logit)
    ACT : e[:, quarter] = exp(ps_logit)  -> e_sb [128, 2048] bf16
    DVE : scalar_tensor_tensor(x*e, accum_out) per chunk [128, 2048]
          (the only engine that can fuse multiply+free-axis reduce;
          Pool rejects TensorScalarPtr at the v3 ISA level)

num partials land in num_parts[128, 32] (col = (b*2+c)*2 + h), one row
of e per half is DMA'd out; host sums partials, computes den = sum(e)
and the final division.  DVE at 1 col/0.96GHz-cycle over 65.5k columns
(~70us) is the critical engine; DMA (~48.5us for the 16.8MB/core bf16
x stream), PE (~43us) and ACT (~43us) hide under it.
"""

import sys
import types

sys.path.insert(0, "/opt/trn_rl_repo")

# bass_utils' trace path imports antenv.axon_hooks, which not every image
# ships; register a no-op fallback so trace=True degrades instead of crashing.
try:
    import antenv.axon_hooks  # noqa: F401
except ImportError:
    try:
        import antenv

        _hooks = types.ModuleType("antenv.axon_hooks")
        _hooks._HOOK = None

        def _set_hook(hook):
            _hooks._HOOK = hook

        def _get_hook():
            return _hooks._HOOK

        _hooks.set_axon_ntff_profile_hook = _set_hook
        _hooks.get_axon_ntff_profile_hook = _get_hook
        sys.modules["antenv.axon_hooks"] = _hooks
        antenv.axon_hooks = _hooks
    except ImportError:
        pass

import numpy as np
import ml_dtypes

import concourse.bacc as bacc
import concourse.tile as tile
from concourse import mybir
from concourse import bass_utils
from concourse.dve_ops import TENSOR_TENSOR_REDUCE

B, T, D, A = 64, 4096, 256, 50
NCORES = 8
BL = B // NCORES  # batches per core
EPS = 1e-7
P = 128
NCH = D // P      # 2 d-chunks
TH = T // 2       # 2048: half, the pipeline granule
TQ = T // 4       # 1024: quarter (psum granule)
NH = 2 * BL       # 16 halves per core
NPART = 1         # one numerator slot per (b, c)
OFF_Z = 1280      # chunk-1 tail columns offloaded to DVE-TT(2x) + ACT reduce
NSLOT = NCH * BL + (BL - 1)  # TTR slots + ACT-offload slots (batches 1..7)


def build_attpool(nc, aps):
    xt, w, bb2, urep = aps["xt"], aps["w"], aps["bb2"], aps["urep"]
    nump, eout = aps["nump"], aps["eout"]
    f32 = mybir.dt.float32
    bf16 = mybir.dt.bfloat16
    LOOKAHEAD = 6

    with tile.TileContext(nc) as tc:
        with (
            tc.tile_pool(name="singles", bufs=1) as singles,
            tc.tile_pool(name="x0", bufs=5) as x0_pool,
            tc.tile_pool(name="x1", bufs=5) as x1_pool,
            tc.tile_pool(name="uitT", bufs=2) as uitT_pool,
            tc.tile_pool(name="e", bufs=3) as e_pool,
            tc.tile_pool(name="scrd", bufs=2) as scrd_pool,
            tc.tile_pool(name="ps_uitT", bufs=2, space="PSUM") as ps_uitT_pool,
            tc.tile_pool(name="ps_logit", bufs=2, space="PSUM") as ps_logit_pool,
        ):
            # constants + persistent outputs (DMAs interleaved with the
            # first x loads below so xt(0) dispatches first)
            w_sb = [
                singles.tile([P, A], bf16, tag=f"w{c}", name=f"w_sb{c}")
                for c in range(NCH)
            ]
            bb2_sb = singles.tile([P, 1], f32)
            urep_sb = singles.tile([P, P], bf16)
            num_parts = singles.tile([P, NSLOT], f32)
            # zeros for act-table preload + PE p-state warmup
            wz = singles.tile([P, 512], bf16)
            wz_out = singles.tile([P, 16], bf16)
            nc.gpsimd.memset(wz[:, :], 0.0)

            xt_tiles = {}   # batch -> [tile_c0, tile_c1], each [P, T]
            e_tiles = {}    # batch -> e tile [P, T]

            def load_batch(b, split=False):
                xt_t = []
                for c, pool in ((0, x0_pool), (1, x1_pool)):
                    tl = pool.tile([P, T], bf16, tag=f"xt{c}", name=f"xt{c}_{b}")
                    xt_t.append(tl)
                if split:
                    # halves land separately so mm1 of h0 starts sooner
                    for h in range(2):
                        for c in range(NCH):
                            nc.sync.dma_start(
                                out=xt_t[c][:, h * TH : (h + 1) * TH],
                                in_=xt[c, :, b * T + h * TH : b * T + (h + 1) * TH],
                            )
                else:
                    for c in range(NCH):
                        nc.sync.dma_start(
                            out=xt_t[c][:, :], in_=xt[c, :, b * T : (b + 1) * T]
                        )
                xt_tiles[b] = xt_t

            def stage1(i):
                """mm1 for half i -> ps_uitT [128, 1024]; quarter q on rows
                64*q..64*q+49 (PE out base partition must be 0/32/64)."""
                b, h = divmod(i, 2)
                xt_t = xt_tiles[b]
                off = h * TH
                ps = ps_uitT_pool.tile([P, TQ], f32, tag="psu")
                for c in range(NCH):  # stationary-major: 1 LDW per chunk
                    for q in range(2):
                        for s in (0, 512):
                            nc.tensor.matmul(
                                ps[64 * q : 64 * q + A, s : s + 512],
                                lhsT=w_sb[c][:, :],
                                rhs=xt_t[c][:, off + q * TQ + s : off + q * TQ + s + 512],
                                start=(c == 0),
                                stop=(c == NCH - 1),
                            )
                return ps

            def stage2a(i, ps_uitT):
                """tanh, mm2, exp for half i -> e tile cols.  The first half
                runs quarter-granular (split tanh, exp per quarter) so the
                first TTR starts as early as possible."""
                b, h = divmod(i, 2)
                if h == 0:
                    e_tiles[b] = e_pool.tile([P, T], bf16, tag="e", name=f"e_{b}")
                e_sb = e_tiles[b]
                off = h * TH
                uitT_sb = uitT_pool.tile([P, TQ], bf16, tag="uitT")
                nc.scalar.activation(
                    uitT_sb[0 : 64 + A, :], ps_uitT[0 : 64 + A, :],
                    mybir.ActivationFunctionType.Tanh,
                    bias=bb2_sb[0 : 64 + A, :],
                )
                for q in range(2):
                    ps_logit = ps_logit_pool.tile([P, TQ], f32, tag="psl")
                    for s in (0, 512):
                        nc.tensor.matmul(
                            ps_logit[:, s : s + 512],
                            lhsT=urep_sb[64 * q : 64 * q + A, :],
                            rhs=uitT_sb[64 * q : 64 * q + A, s : s + 512],
                            start=True,
                            stop=True,
                        )
                    nc.scalar.activation(
                        e_sb[:, off + q * TQ : off + (q + 1) * TQ], ps_logit[:, :],
                        mybir.ActivationFunctionType.Exp,
                    )
                if h == 1:
                    # one (identical) row of e out for the host denominator
                    nc.sync.dma_start(out=eout[b : b + 1, :], in_=e_sb[0:1, :])

            act_red = []  # pending ACT tail-reductions, emitted one step late

            def stage2b(b, half=None):
                """numerator TTRs: accum_out = sum_t x*e.  half=None does the
                whole batch; half=h does one half with s0-chained accum (used
                for batch 0 so the DVE stream starts a half earlier).  For
                full batches, the last OFF_Z columns of chunk 1 go through
                DVE tensor_tensor (2x bf16 mode) with the reduction done on
                the idle ACT engine instead of the critical DVE stream."""
                xt_t = xt_tiles[b]
                e_sb = e_tiles[b]
                segs = [(0, T)] if half is None else [(half * TH, (half + 1) * TH)]
                offload = half is None and OFF_Z > 0
                for c in range(NCH):
                    slot = b * NCH + c
                    acc = num_parts[:, slot : slot + 1]
                    scr = scrd_pool.tile([P, T], bf16, tag="scrd")
                    first = half is None or half == 0
                    for k, (lo, hi) in enumerate(segs):
                        if offload and c == NCH - 1:
                            hi -= OFF_Z
                        nc.vector._custom_dve(
                            TENSOR_TENSOR_REDUCE,
                            out=scr[:, lo:hi],
                            in0=xt_t[c][:, lo:hi],
                            in1=e_sb[:, lo:hi],
                            s0=0.0 if (first and k == 0) else acc,
                            s1=1.0,
                            accum_out=acc,
                        )
                if offload:
                    prod = scrd_pool.tile([P, OFF_Z], bf16, tag="prod")
                    nc.vector.tensor_tensor(
                        out=prod[:, :],
                        in0=xt_t[NCH - 1][:, T - OFF_Z :],
                        in1=e_sb[:, T - OFF_Z :],
                        op=mybir.AluOpType.mult,
                    )
                    act_red.append((b, prod))
                if half is None or half == 1:
                    del xt_tiles[b]
                    del e_tiles[b]

            def flush_act_red():
                while act_red:
                    ab, prod = act_red.pop(0)
                    slot = NCH * BL + (ab - 1)
                    scr3 = scrd_pool.tile([P, OFF_Z], bf16, tag="actred")
                    nc.scalar.activation(
                        scr3[:, :], prod[:, :],
                        mybir.ActivationFunctionType.Copy,
                        accum_out=num_parts[:, slot : slot + 1],
                    )

            # batch-0 loads fan out over four dispatch queues so several DMA
            # transfers are in flight at once (per-transfer bandwidth is a
            # few engines only; aggregate needs concurrency)
            b0 = []
            for c, pool in ((0, x0_pool), (1, x1_pool)):
                b0.append(pool.tile([P, T], bf16, tag=f"xt{c}", name=f"xt{c}_0"))
            xt_tiles[0] = b0
            # critical half-0 as 4x256KB column-split transfers: DMA
            # bandwidth is shared per-transfer (more transfers = bigger
            # share), and the column split unblocks quarter-0 compute early
            for c in range(NCH):
                for r in (0, 64):
                    nc.sync.dma_start(
                        out=b0[c][r : r + 64, 0:TH],
                        in_=xt[c, r : r + 64, 0:TH],
                    )
            for c in range(NCH):
                nc.sync.dma_start(out=w_sb[c][:, :], in_=w[c, :, :])
            nc.sync.dma_start(out=bb2_sb[:, :], in_=bb2[:, :])
            nc.sync.dma_start(out=urep_sb[:, :], in_=urep[:, :])
            nc.sync.dma_start(out=b0[0][:, TH:T], in_=xt[0, :, TH:T])
            nc.sync.dma_start(out=b0[1][:, TH:T], in_=xt[1, :, TH:T])
            # b1 row-split for a bigger bandwidth share during the fill
            b1 = []
            for c, pool in ((0, x0_pool), (1, x1_pool)):
                b1.append(pool.tile([P, T], bf16, tag=f"xt{c}", name=f"xt{c}_1"))
            xt_tiles[1] = b1
            for c in range(NCH):
                for r in (0, 64):
                    nc.sync.dma_start(
                        out=b1[c][r : r + 64, :], in_=xt[c, r : r + 64, T : 2 * T]
                    )
            load_batch(2)
            # preload tanh/exp activation tables off the critical path
            nc.scalar.activation(
                wz_out[:, :], wz[:, 0:16],
                mybir.ActivationFunctionType.Tanh,
            )
            # PE p-state warmup: harmless zero matmuls while x streams in
            # (few enough not to block mm1(0) in the in-order PE queue)
            ps_warm = ps_uitT_pool.tile([P, TQ], f32, tag="psu")
            for r in range(6):
                nc.tensor.matmul(
                    ps_warm[0:64, 0:512],
                    lhsT=wz[:, 0:64],
                    rhs=wz[:, :],
                    start=True,
                    stop=True,
                )

            NB = BL
            pend_a = None  # (i, ps) waiting for stage2a
            done_a = -1    # highest half index with stage2a emitted
            next_b = 0     # next batch to run stage2b
            b0_h0 = False  # batch-0 half-0 TTR emitted
            KD = (NB - 1) * NCH  # slots drained early (all but last batch)
            drained = [False]

            def maybe_drain():
                # drain finished batches' numerators early so only the last
                # batch's slots remain for the tail DMA
                if next_b == NB - 1 and not drained[0]:
                    nc.sync.dma_start(out=nump[:, 0:KD], in_=num_parts[:, 0:KD])
                    drained[0] = True
            for i in range(2 * NB):
                ps = stage1(i)
                if i % 2 == 1 and (i + 1) // 2 + 2 < NB:
                    load_batch((i + 1) // 2 + 2)
                if pend_a is not None:
                    stage2a(*pend_a)
                    done_a = pend_a[0]
                    flush_act_red()
                pend_a = (i, ps)
                # batch 0 streams per half; later batches per whole batch
                if next_b == 0:
                    if not b0_h0 and done_a >= 0:
                        stage2b(0, half=0)
                        b0_h0 = True
                    if b0_h0 and done_a >= 1:
                        stage2b(0, half=1)
                        next_b = 1
                elif done_a >= 2 * next_b + 1:
                    stage2b(next_b)
                    next_b += 1
                    maybe_drain()
            stage2a(*pend_a)
            if next_b == 0:
                if not b0_h0:
                    stage2b(0, half=0)
                stage2b(0, half=1)
                next_b = 1
            while next_b < NB:
                stage2b(next_b)
                next_b += 1
                maybe_drain()
            flush_act_red()
            nc.sync.dma_start(out=nump[:, KD:], in_=num_parts[:, KD:])
    return nc


def _declare(nc):
    f32 = mybir.dt.float32
    bf16 = mybir.dt.bfloat16
    aps = {
        "xt": nc.dram_tensor("xt", (NCH, P, BL * T), bf16, kind="ExternalInput").ap(),
        "w": nc.dram_tensor("w", (NCH, P, A), bf16, kind="ExternalInput").ap(),
        "bb2": nc.dram_tensor("bb2", (P, 1), f32, kind="ExternalInput").ap(),
        "urep": nc.dram_tensor("urep", (P, P), bf16, kind="ExternalInput").ap(),
        "nump": nc.dram_tensor(
            "nump", (P, NSLOT), f32, kind="ExternalOutput"
        ).ap(),
        "eout": nc.dram_tensor("eout", (BL, T), bf16, kind="ExternalOutput").ap(),
    }
    return aps


_CACHE = {}


def _get_nc():
    key = "nc"
    if key not in _CACHE:
        nc = bacc.Bacc(
            "TRN2", target_bir_lowering=False, debug=False,
            enable_asserts=False, num_devices=NCORES,
        )
        aps = _declare(nc)
        build_attpool(nc, aps)
        nc.compile()
        _CACHE[key] = nc
    return _CACHE[key]


def _host_prep(x, W, b, u):
    """Build per-core input maps from full inputs (layout/dtype prep only)."""
    x = np.asarray(x, dtype=np.float32)
    W = np.asarray(W, dtype=np.float32)
    b = np.asarray(b, dtype=np.float32)
    u = np.asarray(u, dtype=np.float32)
    wc = np.ascontiguousarray(W.reshape(NCH, P, A)).astype(ml_dtypes.bfloat16)
    bb2 = np.zeros((P, 1), dtype=np.float32)
    bb2[0:A, 0] = b
    bb2[64 : 64 + A, 0] = b
    urep = np.zeros((P, P), dtype=np.float32)
    urep[0:A, :] = u.reshape(A, 1)
    urep[64 : 64 + A, :] = u.reshape(A, 1)
    urep = np.ascontiguousarray(urep).astype(ml_dtypes.bfloat16)
    in_maps = []
    for core in range(NCORES):
        xc = x[core * BL : (core + 1) * BL]  # [BL, T, D]
        # -> [NCH, P, BL*T]: xt[c, dp, b*T+t] = x[b, t, c*128+dp]
        xt = np.ascontiguousarray(
            xc.reshape(BL, T, NCH, P).transpose(2, 3, 0, 1).reshape(NCH, P, BL * T)
        ).astype(ml_dtypes.bfloat16)
        in_maps.append({"xt": xt, "w": wc, "bb2": bb2, "urep": urep})
    return in_maps


def _unshard(results):
    out = np.empty((B, D), dtype=np.float32)
    for core in range(NCORES):
        nump = results[core]["nump"]              # [128, NSLOT] f32
        eout = np.asarray(results[core]["eout"])  # [BL, T] bf16
        parts = nump[:, : BL * NCH].reshape(P, BL, NCH).copy()
        for bl in range(1, BL):                   # add ACT-offload partials
            parts[:, bl, NCH - 1] += nump[:, NCH * BL + bl - 1]
        den = eout.astype(np.float32).sum(axis=1)  # [BL]
        for bl in range(BL):
            vec = np.concatenate([parts[:, bl, 0], parts[:, bl, 1]])  # [D]
            out[core * BL + bl] = vec / (den[bl] + EPS)
    return out


def kernel(x, W, b, u, _trace=False):
    nc = _get_nc()
    in_maps = _host_prep(x, W, b, u)
    res = bass_utils.run_bass_kernel_spmd(
        nc, in_maps, core_ids=list(range(NCORES)), trace=_trace,
    )
    out = _unshard(res.results)
    if _trace:
        kernel.last_result = res
    return out


# revision 65
# speedup vs baseline: 1.1325x; 1.1325x over previous
"""Attention-pooling kernel (AttLayer) for Trainium2, 8 NeuronCores.

Math (per batch b):
    uit  = tanh(x @ W + b)          # [T, A]
    e    = exp(uit @ u)             # [T]
    out  = (sum_t e[t] * x[t,:]) / (sum_t e[t] + EPS)   # [D]

Per-core structure (pure data parallel over batch, BL=8 batches/core),
processing halves of T (TH=2048) so every engine streams concurrently:

    PE  : ps_uitT[100, 1024] = W^T @ x-half   (both 1024-quarters of the
          half packed on the partition axis: q0 -> rows 0-49, q1 -> 50-99;
          matmuls grouped per stationary: LDW w0 x4MM, LDW w1 x4MM)
    ACT : uitT = tanh(ps_uitT + bias2) as ONE [100, 1024] instr
    PE  : ps_logit[128, 1024] = urep^T @ uitT[rows]  per quarter
          (128 identical rows = partition-broadcast of the logit)
    ACT : e[:, quarter] = exp(ps_logit)  -> e_sb [128, 2048] bf16
    DVE : scalar_tensor_tensor(x*e, accum_out) per chunk [128, 2048]
          (the only engine that can fuse multiply+free-axis reduce;
          Pool rejects TensorScalarPtr at the v3 ISA level)

num partials land in num_parts[128, 32] (col = (b*2+c)*2 + h), one row
of e per half is DMA'd out; host sums partials, computes den = sum(e)
and the final division.  DVE at 1 col/0.96GHz-cycle over 65.5k columns
(~70us) is the critical engine; DMA (~48.5us for the 16.8MB/core bf16
x stream), PE (~43us) and ACT (~43us) hide under it.
"""

import sys
import types

sys.path.insert(0, "/opt/trn_rl_repo")

# bass_utils' trace path imports antenv.axon_hooks, which not every image
# ships; register a no-op fallback so trace=True degrades instead of crashing.
try:
    import antenv.axon_hooks  # noqa: F401
except ImportError:
    try:
        import antenv

        _hooks = types.ModuleType("antenv.axon_hooks")
        _hooks._HOOK = None

        def _set_hook(hook):
            _hooks._HOOK = hook

        def _get_hook():
            return _hooks._HOOK

        _hooks.set_axon_ntff_profile_hook = _set_hook
        _hooks.get_axon_ntff_profile_hook = _get_hook
        sys.modules["antenv.axon_hooks"] = _hooks
        antenv.axon_hooks = _hooks
    except ImportError:
        pass

import numpy as np
import ml_dtypes

import concourse.bacc as bacc
import concourse.tile as tile
from concourse import mybir
from concourse import bass_utils
from concourse.dve_ops import TENSOR_TENSOR_REDUCE

B, T, D, A = 64, 4096, 256, 50
NCORES = 8
BL = B // NCORES  # batches per core
EPS = 1e-7
P = 128
NCH = D // P      # 2 d-chunks
TH = T // 2       # 2048: half, the pipeline granule
TQ = T // 4       # 1024: quarter (psum granule)
NH = 2 * BL       # 16 halves per core
NPART = 1         # one numerator slot per (b, c)
OFF_Z = 704       # chunk-1 tail columns offloaded to Pool-TT + ACT reduce
NSLOT = NCH * BL + (BL - 1)  # TTR slots + ACT-offload slots (batches 1..7)


def build_attpool(nc, aps):
    xt, w, bb2, urep = aps["xt"], aps["w"], aps["bb2"], aps["urep"]
    nump, eout = aps["nump"], aps["eout"]
    f32 = mybir.dt.float32
    bf16 = mybir.dt.bfloat16
    LOOKAHEAD = 6

    with tile.TileContext(nc) as tc:
        with (
            tc.tile_pool(name="singles", bufs=1) as singles,
            tc.tile_pool(name="x0", bufs=5) as x0_pool,
            tc.tile_pool(name="x1", bufs=5) as x1_pool,
            tc.tile_pool(name="uitT", bufs=2) as uitT_pool,
            tc.tile_pool(name="e", bufs=3) as e_pool,
            tc.tile_pool(name="scrd", bufs=4) as scrd_pool,
            tc.tile_pool(name="ps_uitT", bufs=2, space="PSUM") as ps_uitT_pool,
            tc.tile_pool(name="ps_logit", bufs=2, space="PSUM") as ps_logit_pool,
        ):
            # constants + persistent outputs (DMAs interleaved with the
            # first x loads below so xt(0) dispatches first)
            w_sb = [
                singles.tile([P, A], bf16, tag=f"w{c}", name=f"w_sb{c}")
                for c in range(NCH)
            ]
            bb2_sb = singles.tile([P, 1], f32)
            urep_sb = singles.tile([P, P], bf16)
            num_parts = singles.tile([P, NSLOT], f32)
            # zeros for act-table preload + PE p-state warmup
            wz = singles.tile([P, 512], bf16)
            wz_out = singles.tile([P, 16], bf16)
            nc.gpsimd.memset(wz[:, :], 0.0)

            xt_tiles = {}   # batch -> [tile_c0, tile_c1], each [P, T]
            e_tiles = {}    # batch -> e tile [P, T]

            def load_batch(b, split=False):
                xt_t = []
                for c, pool in ((0, x0_pool), (1, x1_pool)):
                    tl = pool.tile([P, T], bf16, tag=f"xt{c}", name=f"xt{c}_{b}")
                    xt_t.append(tl)
                if split:
                    # halves land separately so mm1 of h0 starts sooner
                    for h in range(2):
                        for c in range(NCH):
                            nc.sync.dma_start(
                                out=xt_t[c][:, h * TH : (h + 1) * TH],
                                in_=xt[c, :, b * T + h * TH : b * T + (h + 1) * TH],
                            )
                else:
                    for c in range(NCH):
                        nc.sync.dma_start(
                            out=xt_t[c][:, :], in_=xt[c, :, b * T : (b + 1) * T]
                        )
                xt_tiles[b] = xt_t

            def stage1(i):
                """mm1 for half i -> ps_uitT [128, 1024]; quarter q on rows
                64*q..64*q+49 (PE out base partition must be 0/32/64)."""
                b, h = divmod(i, 2)
                xt_t = xt_tiles[b]
                off = h * TH
                ps = ps_uitT_pool.tile([P, TQ], f32, tag="psu")
                for c in range(NCH):  # stationary-major: 1 LDW per chunk
                    for q in range(2):
                        for s in (0, 512):
                            nc.tensor.matmul(
                                ps[64 * q : 64 * q + A, s : s + 512],
                                lhsT=w_sb[c][:, :],
                                rhs=xt_t[c][:, off + q * TQ + s : off + q * TQ + s + 512],
                                start=(c == 0),
                                stop=(c == NCH - 1),
                            )
                return ps

            def stage2a(i, ps_uitT):
                """tanh, mm2, exp for half i -> e tile cols.  The first half
                runs quarter-granular (split tanh, exp per quarter) so the
                first TTR starts as early as possible."""
                b, h = divmod(i, 2)
                if h == 0:
                    e_tiles[b] = e_pool.tile([P, T], bf16, tag="e", name=f"e_{b}")
                e_sb = e_tiles[b]
                off = h * TH
                uitT_sb = uitT_pool.tile([P, TQ], bf16, tag="uitT")
                nc.scalar.activation(
                    uitT_sb[0 : 64 + A, :], ps_uitT[0 : 64 + A, :],
                    mybir.ActivationFunctionType.Tanh,
                    bias=bb2_sb[0 : 64 + A, :],
                )
                for q in range(2):
                    ps_logit = ps_logit_pool.tile([P, TQ], f32, tag="psl")
                    for s in (0, 512):
                        nc.tensor.matmul(
                            ps_logit[:, s : s + 512],
                            lhsT=urep_sb[64 * q : 64 * q + A, :],
                            rhs=uitT_sb[64 * q : 64 * q + A, s : s + 512],
                            start=True,
                            stop=True,
                        )
                    nc.scalar.activation(
                        e_sb[:, off + q * TQ : off + (q + 1) * TQ], ps_logit[:, :],
                        mybir.ActivationFunctionType.Exp,
                    )
                if h == 1:
                    # one (identical) row of e out for the host denominator
                    nc.sync.dma_start(out=eout[b : b + 1, :], in_=e_sb[0:1, :])

            act_red = []  # pending ACT tail-reductions, emitted one step late

            def stage2b(b, half=None):
                """numerator TTRs: accum_out = sum_t x*e.  half=None does the
                whole batch; half=h does one half with s0-chained accum (used
                for batch 0 so the DVE stream starts a half earlier).  For
                full batches, the last OFF_Z columns of chunk 1 go through
                DVE tensor_tensor (2x bf16 mode) with the reduction done on
                the idle ACT engine instead of the critical DVE stream."""
                xt_t = xt_tiles[b]
                e_sb = e_tiles[b]
                segs = [(0, T)] if half is None else [(half * TH, (half + 1) * TH)]
                offload = half is None and OFF_Z > 0
                for c in range(NCH):
                    slot = b * NCH + c
                    acc = num_parts[:, slot : slot + 1]
                    scr = scrd_pool.tile([P, T], bf16, tag="scrd")
                    first = half is None or half == 0
                    for k, (lo, hi) in enumerate(segs):
                        if offload and c == NCH - 1:
                            hi -= OFF_Z
                        nc.vector._custom_dve(
                            TENSOR_TENSOR_REDUCE,
                            out=scr[:, lo:hi],
                            in0=xt_t[c][:, lo:hi],
                            in1=e_sb[:, lo:hi],
                            s0=0.0 if (first and k == 0) else acc,
                            s1=1.0,
                            accum_out=acc,
                        )
                if offload:
                    prod = scrd_pool.tile([P, OFF_Z], bf16, tag="prod")
                    nc.gpsimd.tensor_tensor(
                        out=prod[:, :],
                        in0=xt_t[NCH - 1][:, T - OFF_Z :],
                        in1=e_sb[:, T - OFF_Z :],
                        op=mybir.AluOpType.mult,
                    )
                    act_red.append((b, prod))
                if half is None or half == 1:
                    del xt_tiles[b]
                    del e_tiles[b]

            def flush_act_red(keep=0):
                while len(act_red) > keep:
                    ab, prod = act_red.pop(0)
                    slot = NCH * BL + (ab - 1)
                    scr3 = scrd_pool.tile([P, OFF_Z], bf16, tag="actred")
                    nc.scalar.activation(
                        scr3[:, :], prod[:, :],
                        mybir.ActivationFunctionType.Copy,
                        accum_out=num_parts[:, slot : slot + 1],
                    )

            # batch-0 loads fan out over four dispatch queues so several DMA
            # transfers are in flight at once (per-transfer bandwidth is a
            # few engines only; aggregate needs concurrency)
            b0 = []
            for c, pool in ((0, x0_pool), (1, x1_pool)):
                b0.append(pool.tile([P, T], bf16, tag=f"xt{c}", name=f"xt{c}_0"))
            xt_tiles[0] = b0
            # critical half-0 as 4x256KB column-split transfers: DMA
            # bandwidth is shared per-transfer (more transfers = bigger
            # share), and the column split unblocks quarter-0 compute early
            for c in range(NCH):
                for r in (0, 64):
                    nc.sync.dma_start(
                        out=b0[c][r : r + 64, 0:TH],
                        in_=xt[c, r : r + 64, 0:TH],
                    )
            for c in range(NCH):
                nc.sync.dma_start(out=w_sb[c][:, :], in_=w[c, :, :])
            nc.sync.dma_start(out=bb2_sb[:, :], in_=bb2[:, :])
            nc.sync.dma_start(out=urep_sb[:, :], in_=urep[:, :])
            nc.sync.dma_start(out=b0[0][:, TH:T], in_=xt[0, :, TH:T])
            nc.sync.dma_start(out=b0[1][:, TH:T], in_=xt[1, :, TH:T])
            # b1 row-split for a bigger bandwidth share during the fill
            b1 = []
            for c, pool in ((0, x0_pool), (1, x1_pool)):
                b1.append(pool.tile([P, T], bf16, tag=f"xt{c}", name=f"xt{c}_1"))
            xt_tiles[1] = b1
            for c in range(NCH):
                for r in (0, 64):
                    nc.sync.dma_start(
                        out=b1[c][r : r + 64, :], in_=xt[c, r : r + 64, T : 2 * T]
                    )
            load_batch(2)
            # preload tanh/exp activation tables off the critical path
            nc.scalar.activation(
                wz_out[:, :], wz[:, 0:16],
                mybir.ActivationFunctionType.Tanh,
            )
            # PE p-state warmup: harmless zero matmuls while x streams in
            # (few enough not to block mm1(0) in the in-order PE queue)
            ps_warm = ps_uitT_pool.tile([P, TQ], f32, tag="psu")
            for r in range(6):
                nc.tensor.matmul(
                    ps_warm[0:64, 0:512],
                    lhsT=wz[:, 0:64],
                    rhs=wz[:, :],
                    start=True,
                    stop=True,
                )

            NB = BL
            pend_a = None  # (i, ps) waiting for stage2a
            done_a = -1    # highest half index with stage2a emitted
            next_b = 0     # next batch to run stage2b
            b0_h0 = False  # batch-0 half-0 TTR emitted
            KD = (NB - 1) * NCH  # slots drained early (all but last batch)
            drained = [False]

            def maybe_drain():
                # drain finished batches' numerators early so only the last
                # batch's slots remain for the tail DMA
                if next_b == NB - 1 and not drained[0]:
                    nc.sync.dma_start(out=nump[:, 0:KD], in_=num_parts[:, 0:KD])
                    drained[0] = True
            for i in range(2 * NB):
                ps = stage1(i)
                if i % 2 == 1 and (i + 1) // 2 + 2 < NB:
                    load_batch((i + 1) // 2 + 2)
                if pend_a is not None:
                    stage2a(*pend_a)
                    done_a = pend_a[0]
                    flush_act_red(keep=2)
                pend_a = (i, ps)
                # batch 0 streams per half; later batches per whole batch
                if next_b == 0:
                    if not b0_h0 and done_a >= 0:
                        stage2b(0, half=0)
                        b0_h0 = True
                    if b0_h0 and done_a >= 1:
                        stage2b(0, half=1)
                        next_b = 1
                elif done_a >= 2 * next_b + 1:
                    stage2b(next_b)
                    next_b += 1
                    maybe_drain()
            stage2a(*pend_a)
            if next_b == 0:
                if not b0_h0:
                    stage2b(0, half=0)
                stage2b(0, half=1)
                next_b = 1
            while next_b < NB:
                stage2b(next_b)
                next_b += 1
                maybe_drain()
            flush_act_red()
            nc.sync.dma_start(out=nump[:, KD:], in_=num_parts[:, KD:])
    return nc


def _declare(nc):
    f32 = mybir.dt.float32
    bf16 = mybir.dt.bfloat16
    aps = {
        "xt": nc.dram_tensor("xt", (NCH, P, BL * T), bf16, kind="ExternalInput").ap(),
        "w": nc.dram_tensor("w", (NCH, P, A), bf16, kind="ExternalInput").ap(),
        "bb2": nc.dram_tensor("bb2", (P, 1), f32, kind="ExternalInput").ap(),
        "urep": nc.dram_tensor("urep", (P, P), bf16, kind="ExternalInput").ap(),
        "nump": nc.dram_tensor(
            "nump", (P, NSLOT), f32, kind="ExternalOutput"
        ).ap(),
        "eout": nc.dram_tensor("eout", (BL, T), bf16, kind="ExternalOutput").ap(),
    }
    return aps


_CACHE = {}


def _get_nc():
    key = "nc"
    if key not in _CACHE:
        nc = bacc.Bacc(
            "TRN2", target_bir_lowering=False, debug=False,
            enable_asserts=False, num_devices=NCORES,
        )
        aps = _declare(nc)
        build_attpool(nc, aps)
        nc.compile()
        _CACHE[key] = nc
    return _CACHE[key]


def _host_prep(x, W, b, u):
    """Build per-core input maps from full inputs (layout/dtype prep only)."""
    x = np.asarray(x, dtype=np.float32)
    W = np.asarray(W, dtype=np.float32)
    b = np.asarray(b, dtype=np.float32)
    u = np.asarray(u, dtype=np.float32)
    wc = np.ascontiguousarray(W.reshape(NCH, P, A)).astype(ml_dtypes.bfloat16)
    bb2 = np.zeros((P, 1), dtype=np.float32)
    bb2[0:A, 0] = b
    bb2[64 : 64 + A, 0] = b
    urep = np.zeros((P, P), dtype=np.float32)
    urep[0:A, :] = u.reshape(A, 1)
    urep[64 : 64 + A, :] = u.reshape(A, 1)
    urep = np.ascontiguousarray(urep).astype(ml_dtypes.bfloat16)
    in_maps = []
    for core in range(NCORES):
        xc = x[core * BL : (core + 1) * BL]  # [BL, T, D]
        # -> [NCH, P, BL*T]: xt[c, dp, b*T+t] = x[b, t, c*128+dp]
        xt = np.ascontiguousarray(
            xc.reshape(BL, T, NCH, P).transpose(2, 3, 0, 1).reshape(NCH, P, BL * T)
        ).astype(ml_dtypes.bfloat16)
        in_maps.append({"xt": xt, "w": wc, "bb2": bb2, "urep": urep})
    return in_maps


def _unshard(results):
    out = np.empty((B, D), dtype=np.float32)
    for core in range(NCORES):
        nump = results[core]["nump"]              # [128, NSLOT] f32
        eout = np.asarray(results[core]["eout"])  # [BL, T] bf16
        parts = nump[:, : BL * NCH].reshape(P, BL, NCH).copy()
        for bl in range(1, BL):                   # add ACT-offload partials
            parts[:, bl, NCH - 1] += nump[:, NCH * BL + bl - 1]
        den = eout.astype(np.float32).sum(axis=1)  # [BL]
        for bl in range(BL):
            vec = np.concatenate([parts[:, bl, 0], parts[:, bl, 1]])  # [D]
            out[core * BL + bl] = vec / (den[bl] + EPS)
    return out


def kernel(x, W, b, u, _trace=False):
    nc = _get_nc()
    in_maps = _host_prep(x, W, b, u)
    res = bass_utils.run_bass_kernel_spmd(
        nc, in_maps, core_ids=list(range(NCORES)), trace=_trace,
    )
    out = _unshard(res.results)
    if _trace:
        kernel.last_result = res
    return out


# revision 66
# speedup vs baseline: 1.1384x; 1.0052x over previous
"""Attention-pooling kernel (AttLayer) for Trainium2, 8 NeuronCores.

Math (per batch b):
    uit  = tanh(x @ W + b)          # [T, A]
    e    = exp(uit @ u)             # [T]
    out  = (sum_t e[t] * x[t,:]) / (sum_t e[t] + EPS)   # [D]

Per-core structure (pure data parallel over batch, BL=8 batches/core),
processing halves of T (TH=2048) so every engine streams concurrently:

    PE  : ps_uitT[100, 1024] = W^T @ x-half   (both 1024-quarters of the
          half packed on the partition axis: q0 -> rows 0-49, q1 -> 50-99;
          matmuls grouped per stationary: LDW w0 x4MM, LDW w1 x4MM)
    ACT : uitT = tanh(ps_uitT + bias2) as ONE [100, 1024] instr
    PE  : ps_logit[128, 1024] = urep^T @ uitT[rows]  per quarter
          (128 identical rows = partition-broadcast of the logit)
    ACT : e[:, quarter] = exp(ps_logit)  -> e_sb [128, 2048] bf16
    DVE : scalar_tensor_tensor(x*e, accum_out) per chunk [128, 2048]
          (the only engine that can fuse multiply+free-axis reduce;
          Pool rejects TensorScalarPtr at the v3 ISA level)

num partials land in num_parts[128, 32] (col = (b*2+c)*2 + h), one row
of e per half is DMA'd out; host sums partials, computes den = sum(e)
and the final division.  DVE at 1 col/0.96GHz-cycle over 65.5k columns
(~70us) is the critical engine; DMA (~48.5us for the 16.8MB/core bf16
x stream), PE (~43us) and ACT (~43us) hide under it.
"""

import sys
import types

sys.path.insert(0, "/opt/trn_rl_repo")

# bass_utils' trace path imports antenv.axon_hooks, which not every image
# ships; register a no-op fallback so trace=True degrades instead of crashing.
try:
    import antenv.axon_hooks  # noqa: F401
except ImportError:
    try:
        import antenv

        _hooks = types.ModuleType("antenv.axon_hooks")
        _hooks._HOOK = None

        def _set_hook(hook):
            _hooks._HOOK = hook

        def _get_hook():
            return _hooks._HOOK

        _hooks.set_axon_ntff_profile_hook = _set_hook
        _hooks.get_axon_ntff_profile_hook = _get_hook
        sys.modules["antenv.axon_hooks"] = _hooks
        antenv.axon_hooks = _hooks
    except ImportError:
        pass

import numpy as np
import ml_dtypes

import concourse.bacc as bacc
import concourse.tile as tile
from concourse import mybir
from concourse import bass_utils
from concourse.dve_ops import TENSOR_TENSOR_REDUCE

B, T, D, A = 64, 4096, 256, 50
NCORES = 8
BL = B // NCORES  # batches per core
EPS = 1e-7
P = 128
NCH = D // P      # 2 d-chunks
TH = T // 2       # 2048: half, the pipeline granule
TQ = T // 4       # 1024: quarter (psum granule)
NH = 2 * BL       # 16 halves per core
NPART = 1         # one numerator slot per (b, c)
OFF_Z = 704       # chunk-1 tail columns offloaded to Pool-TT + ACT reduce
NSLOT = NCH * BL + (BL - 1)  # TTR slots + ACT-offload slots (batches 1..7)


def build_attpool(nc, aps):
    xt, w, bb2, urep = aps["xt"], aps["w"], aps["bb2"], aps["urep"]
    nump, eout = aps["nump"], aps["eout"]
    f32 = mybir.dt.float32
    bf16 = mybir.dt.bfloat16
    LOOKAHEAD = 6

    with tile.TileContext(nc) as tc:
        with (
            tc.tile_pool(name="singles", bufs=1) as singles,
            tc.tile_pool(name="x0", bufs=5) as x0_pool,
            tc.tile_pool(name="x1", bufs=5) as x1_pool,
            tc.tile_pool(name="uitT", bufs=2) as uitT_pool,
            tc.tile_pool(name="e", bufs=3) as e_pool,
            tc.tile_pool(name="scrd", bufs=4) as scrd_pool,
            tc.tile_pool(name="ps_uitT", bufs=2, space="PSUM") as ps_uitT_pool,
            tc.tile_pool(name="ps_logit", bufs=2, space="PSUM") as ps_logit_pool,
        ):
            # constants + persistent outputs (DMAs interleaved with the
            # first x loads below so xt(0) dispatches first)
            w_sb = [
                singles.tile([P, A], bf16, tag=f"w{c}", name=f"w_sb{c}")
                for c in range(NCH)
            ]
            bb2_sb = singles.tile([P, 1], f32)
            urep_sb = singles.tile([P, P], bf16)
            num_parts = singles.tile([P, NSLOT], f32)
            # zeros for act-table preload + PE p-state warmup
            wz = singles.tile([P, 512], bf16)
            wz_out = singles.tile([P, 16], bf16)
            nc.gpsimd.memset(wz[:, :], 0.0)

            xt_tiles = {}   # batch -> [tile_c0, tile_c1], each [P, T]
            e_tiles = {}    # batch -> e tile [P, T]

            def load_batch(b, split=False):
                xt_t = []
                for c, pool in ((0, x0_pool), (1, x1_pool)):
                    tl = pool.tile([P, T], bf16, tag=f"xt{c}", name=f"xt{c}_{b}")
                    xt_t.append(tl)
                if split:
                    # halves land separately so mm1 of h0 starts sooner
                    for h in range(2):
                        for c in range(NCH):
                            nc.sync.dma_start(
                                out=xt_t[c][:, h * TH : (h + 1) * TH],
                                in_=xt[c, :, b * T + h * TH : b * T + (h + 1) * TH],
                            )
                else:
                    for c in range(NCH):
                        nc.sync.dma_start(
                            out=xt_t[c][:, :], in_=xt[c, :, b * T : (b + 1) * T]
                        )
                xt_tiles[b] = xt_t

            def stage1(i):
                """mm1 for half i -> ps_uitT [128, 1024]; quarter q on rows
                64*q..64*q+49 (PE out base partition must be 0/32/64)."""
                b, h = divmod(i, 2)
                xt_t = xt_tiles[b]
                off = h * TH
                ps = ps_uitT_pool.tile([P, TQ], f32, tag="psu")
                for c in range(NCH):  # stationary-major: 1 LDW per chunk
                    for q in range(2):
                        for s in (0, 512):
                            nc.tensor.matmul(
                                ps[64 * q : 64 * q + A, s : s + 512],
                                lhsT=w_sb[c][:, :],
                                rhs=xt_t[c][:, off + q * TQ + s : off + q * TQ + s + 512],
                                start=(c == 0),
                                stop=(c == NCH - 1),
                            )
                return ps

            def stage2a(i, ps_uitT):
                """tanh, mm2, exp for half i -> e tile cols.  The first half
                runs quarter-granular (split tanh, exp per quarter) so the
                first TTR starts as early as possible."""
                b, h = divmod(i, 2)
                if h == 0:
                    e_tiles[b] = e_pool.tile([P, T], bf16, tag="e", name=f"e_{b}")
                e_sb = e_tiles[b]
                off = h * TH
                uitT_sb = uitT_pool.tile([P, TQ], bf16, tag="uitT")
                nc.scalar.activation(
                    uitT_sb[0 : 64 + A, :], ps_uitT[0 : 64 + A, :],
                    mybir.ActivationFunctionType.Tanh,
                    bias=bb2_sb[0 : 64 + A, :],
                )
                for q in range(2):
                    ps_logit = ps_logit_pool.tile([P, TQ], f32, tag="psl")
                    for s in (0, 512):
                        nc.tensor.matmul(
                            ps_logit[:, s : s + 512],
                            lhsT=urep_sb[64 * q : 64 * q + A, :],
                            rhs=uitT_sb[64 * q : 64 * q + A, s : s + 512],
                            start=True,
                            stop=True,
                        )
                    nc.scalar.activation(
                        e_sb[:, off + q * TQ : off + (q + 1) * TQ], ps_logit[:, :],
                        mybir.ActivationFunctionType.Exp,
                    )
                if h == 1:
                    # one (identical) row of e out for the host denominator
                    nc.sync.dma_start(out=eout[b : b + 1, :], in_=e_sb[0:1, :])

            act_red = []  # pending ACT tail-reductions, emitted one step late

            def stage2b(b, half=None):
                """numerator TTRs: accum_out = sum_t x*e.  half=None does the
                whole batch; half=h does one half with s0-chained accum (used
                for batch 0 so the DVE stream starts a half earlier).  For
                full batches, the last OFF_Z columns of chunk 1 go through
                DVE tensor_tensor (2x bf16 mode) with the reduction done on
                the idle ACT engine instead of the critical DVE stream."""
                xt_t = xt_tiles[b]
                e_sb = e_tiles[b]
                segs = [(0, T)] if half is None else [(half * TH, (half + 1) * TH)]
                offload = half is None and OFF_Z > 0
                for c in range(NCH):
                    slot = b * NCH + c
                    acc = num_parts[:, slot : slot + 1]
                    scr = scrd_pool.tile([P, T], bf16, tag="scrd")
                    first = half is None or half == 0
                    for k, (lo, hi) in enumerate(segs):
                        if offload and c == NCH - 1:
                            hi -= OFF_Z
                        nc.vector._custom_dve(
                            TENSOR_TENSOR_REDUCE,
                            out=scr[:, lo:hi],
                            in0=xt_t[c][:, lo:hi],
                            in1=e_sb[:, lo:hi],
                            s0=0.0 if (first and k == 0) else acc,
                            s1=1.0,
                            accum_out=acc,
                        )
                if offload:
                    prod = scrd_pool.tile([P, OFF_Z], bf16, tag="prod")
                    nc.gpsimd.tensor_tensor(
                        out=prod[:, :],
                        in0=xt_t[NCH - 1][:, T - OFF_Z :],
                        in1=e_sb[:, T - OFF_Z :],
                        op=mybir.AluOpType.mult,
                    )
                    act_red.append((b, prod))
                if half is None or half == 1:
                    del xt_tiles[b]
                    del e_tiles[b]

            def flush_act_red(keep=0):
                while len(act_red) > keep:
                    ab, prod = act_red.pop(0)
                    slot = NCH * BL + (ab - 1)
                    scr3 = scrd_pool.tile([P, OFF_Z], bf16, tag="actred")
                    nc.scalar.activation(
                        scr3[:, :], prod[:, :],
                        mybir.ActivationFunctionType.Copy,
                        accum_out=num_parts[:, slot : slot + 1],
                    )

            # batch-0 loads fan out over four dispatch queues so several DMA
            # transfers are in flight at once (per-transfer bandwidth is a
            # few engines only; aggregate needs concurrency)
            b0 = []
            for c, pool in ((0, x0_pool), (1, x1_pool)):
                b0.append(pool.tile([P, T], bf16, tag=f"xt{c}", name=f"xt{c}_0"))
            xt_tiles[0] = b0
            # critical half-0 as 4x256KB column-split transfers: DMA
            # bandwidth is shared per-transfer (more transfers = bigger
            # share), and the column split unblocks quarter-0 compute early
            for c in range(NCH):
                for r in (0, 64):
                    nc.sync.dma_start(
                        out=b0[c][r : r + 64, 0:TH],
                        in_=xt[c, r : r + 64, 0:TH],
                    )
            for c in range(NCH):
                nc.sync.dma_start(out=w_sb[c][:, :], in_=w[c, :, :])
            nc.sync.dma_start(out=bb2_sb[:, :], in_=bb2[:, :])
            nc.sync.dma_start(out=urep_sb[:, :], in_=urep[:, :])
            nc.sync.dma_start(out=b0[0][:, TH:T], in_=xt[0, :, TH:T])
            nc.sync.dma_start(out=b0[1][:, TH:T], in_=xt[1, :, TH:T])
            # b1 row-split for a bigger bandwidth share during the fill
            b1 = []
            for c, pool in ((0, x0_pool), (1, x1_pool)):
                b1.append(pool.tile([P, T], bf16, tag=f"xt{c}", name=f"xt{c}_1"))
            xt_tiles[1] = b1
            for c in range(NCH):
                for r in (0, 64):
                    nc.sync.dma_start(
                        out=b1[c][r : r + 64, :], in_=xt[c, r : r + 64, T : 2 * T]
                    )
            load_batch(2)
            # preload tanh/exp activation tables off the critical path
            nc.scalar.activation(
                wz_out[:, :], wz[:, 0:16],
                mybir.ActivationFunctionType.Tanh,
            )
            # PE p-state warmup: harmless zero matmuls while x streams in
            # (few enough not to block mm1(0) in the in-order PE queue)
            ps_warm = ps_uitT_pool.tile([P, TQ], f32, tag="psu")
            for r in range(10):
                nc.tensor.matmul(
                    ps_warm[0:64, 0:512],
                    lhsT=wz[:, 0:64],
                    rhs=wz[:, :],
                    start=True,
                    stop=True,
                )

            NB = BL
            pend_a = None  # (i, ps) waiting for stage2a
            done_a = -1    # highest half index with stage2a emitted
            next_b = 0     # next batch to run stage2b
            b0_h0 = False  # batch-0 half-0 TTR emitted
            KD = (NB - 1) * NCH  # slots drained early (all but last batch)
            drained = [False]

            def maybe_drain():
                # drain finished batches' numerators early so only the last
                # batch's slots remain for the tail DMA
                if next_b == NB - 1 and not drained[0]:
                    nc.sync.dma_start(out=nump[:, 0:KD], in_=num_parts[:, 0:KD])
                    drained[0] = True
            for i in range(2 * NB):
                ps = stage1(i)
                if i % 2 == 1 and (i + 1) // 2 + 2 < NB:
                    load_batch((i + 1) // 2 + 2)
                if pend_a is not None:
                    stage2a(*pend_a)
                    done_a = pend_a[0]
                    flush_act_red(keep=2)
                pend_a = (i, ps)
                # batch 0 streams per half; later batches per whole batch
                if next_b == 0:
                    if not b0_h0 and done_a >= 0:
                        stage2b(0, half=0)
                        b0_h0 = True
                    if b0_h0 and done_a >= 1:
                        stage2b(0, half=1)
                        next_b = 1
                elif done_a >= 2 * next_b + 1:
                    stage2b(next_b)
                    next_b += 1
                    maybe_drain()
            stage2a(*pend_a)
            if next_b == 0:
                if not b0_h0:
                    stage2b(0, half=0)
                stage2b(0, half=1)
                next_b = 1
            while next_b < NB:
                stage2b(next_b)
                next_b += 1
                maybe_drain()
            flush_act_red()
            nc.sync.dma_start(out=nump[:, KD:], in_=num_parts[:, KD:])
    return nc


def _declare(nc):
    f32 = mybir.dt.float32
    bf16 = mybir.dt.bfloat16
    aps = {
        "xt": nc.dram_tensor("xt", (NCH, P, BL * T), bf16, kind="ExternalInput").ap(),
        "w": nc.dram_tensor("w", (NCH, P, A), bf16, kind="ExternalInput").ap(),
        "bb2": nc.dram_tensor("bb2", (P, 1), f32, kind="ExternalInput").ap(),
        "urep": nc.dram_tensor("urep", (P, P), bf16, kind="ExternalInput").ap(),
        "nump": nc.dram_tensor(
            "nump", (P, NSLOT), f32, kind="ExternalOutput"
        ).ap(),
        "eout": nc.dram_tensor("eout", (BL, T), bf16, kind="ExternalOutput").ap(),
    }
    return aps


_CACHE = {}


def _get_nc():
    key = "nc"
    if key not in _CACHE:
        nc = bacc.Bacc(
            "TRN2", target_bir_lowering=False, debug=False,
            enable_asserts=False, num_devices=NCORES,
        )
        aps = _declare(nc)
        build_attpool(nc, aps)
        nc.compile()
        _CACHE[key] = nc
    return _CACHE[key]


def _host_prep(x, W, b, u):
    """Build per-core input maps from full inputs (layout/dtype prep only)."""
    x = np.asarray(x, dtype=np.float32)
    W = np.asarray(W, dtype=np.float32)
    b = np.asarray(b, dtype=np.float32)
    u = np.asarray(u, dtype=np.float32)
    wc = np.ascontiguousarray(W.reshape(NCH, P, A)).astype(ml_dtypes.bfloat16)
    bb2 = np.zeros((P, 1), dtype=np.float32)
    bb2[0:A, 0] = b
    bb2[64 : 64 + A, 0] = b
    urep = np.zeros((P, P), dtype=np.float32)
    urep[0:A, :] = u.reshape(A, 1)
    urep[64 : 64 + A, :] = u.reshape(A, 1)
    urep = np.ascontiguousarray(urep).astype(ml_dtypes.bfloat16)
    in_maps = []
    for core in range(NCORES):
        xc = x[core * BL : (core + 1) * BL]  # [BL, T, D]
        # -> [NCH, P, BL*T]: xt[c, dp, b*T+t] = x[b, t, c*128+dp]
        xt = np.ascontiguousarray(
            xc.reshape(BL, T, NCH, P).transpose(2, 3, 0, 1).reshape(NCH, P, BL * T)
        ).astype(ml_dtypes.bfloat16)
        in_maps.append({"xt": xt, "w": wc, "bb2": bb2, "urep": urep})
    return in_maps


def _unshard(results):
    out = np.empty((B, D), dtype=np.float32)
    for core in range(NCORES):
        nump = results[core]["nump"]              # [128, NSLOT] f32
        eout = np.asarray(results[core]["eout"])  # [BL, T] bf16
        parts = nump[:, : BL * NCH].reshape(P, BL, NCH).copy()
        for bl in range(1, BL):                   # add ACT-offload partials
            parts[:, bl, NCH - 1] += nump[:, NCH * BL + bl - 1]
        den = eout.astype(np.float32).sum(axis=1)  # [BL]
        for bl in range(BL):
            vec = np.concatenate([parts[:, bl, 0], parts[:, bl, 1]])  # [D]
            out[core * BL + bl] = vec / (den[bl] + EPS)
    return out


def kernel(x, W, b, u, _trace=False):
    nc = _get_nc()
    in_maps = _host_prep(x, W, b, u)
    res = bass_utils.run_bass_kernel_spmd(
        nc, in_maps, core_ids=list(range(NCORES)), trace=_trace,
    )
    out = _unshard(res.results)
    if _trace:
        kernel.last_result = res
    return out
